# revision 33
# baseline (speedup 1.0000x reference)
"""Trainium2 Bass kernel for nn_Block_24111946399747 (dense transformer block).

Three balanced SPMD launches (host resharding between launches; no on-device
collectives available on this terminal):

L1 (row-parallel; core i owns tokens 512i..512i+512):
  - LayerNorm in feature-major layout from a host-pretransposed x^T slice
    (LN affine folded into the QKV weights on host).  Stats (sum, sum-sq)
    via ones-matmuls over the partition dim; normalize on DVE.  Zero
    transposes anywhere.
  - QKV: q^T,k^T feature-major [4096f, 512t] (lhsT = W chunks) and v
    token-major [512t, 2048f] (lhsT = h^T chunks).

L2 (head-parallel attention; core i owns heads 2i, 2i+1; causal work is
  identical per core):
      S^T  [tk, tq] = k_t.T @ q_t ;  P^T = exp(S^T/sqrt(HD)) (causal)
      y_t  [HD, tq] = v^T @ P^T    (rowsum via ones-matmul, then scale)
  q/k/v arrive pre-laid-out from the host; returns y_t [256, 4096] bf16.

L3 (row-parallel proj+MLP; core i owns tokens 512i..512i+512):
  - x2^T = w_proj^T @ Y^T + x^T + b  (feature-major, rhs = Y^T)
  - m^T  = gelu(w_fc^T @ x2^T + b)   (feature-major)
  - out  = m @ w_out + x2 + b        (token-major; lhsT = m^T slices; the
    x2 residual is folded in as identity matmuls, so no transpose is needed)

Matmuls in bf16 with fp32 PSUM accumulation; LN stats, softmax sums in fp32.
"""

import math
import sys

import numpy as np

if "/opt/trn_rl_repo" not in sys.path:
    sys.path.insert(0, "/opt/trn_rl_repo")

import ml_dtypes  # noqa: E402

import concourse.bass as bass  # noqa: E402,F401
import concourse.mybir as mybir  # noqa: E402
import concourse.tile as tile  # noqa: E402
from concourse import bacc  # noqa: E402
from concourse.bass_utils import run_bass_kernel_spmd  # noqa: E402

B, T, C, H = 2, 2048, 2048, 16
HD = C // H            # 128 head dim
N_CORES = 8
HPC = H // N_CORES     # 2 heads per core
TOK = B * T            # 4096 tokens
RPC = TOK // N_CORES   # 512 rows (tokens) per core
P = 128
KC = C // P            # 16 contraction chunks over C
F4 = 4 * C             # 8192
MC4 = F4 // P          # 64 feature chunks over 4C
NT = TOK // P          # 32 token tiles of 128
JTT = T // 512         # 4 query tiles of 512 per batch
EPS = 1e-6
BF16 = mybir.dt.bfloat16
F32 = mybir.dt.float32
ACT = mybir.ActivationFunctionType
ALU_MULT = mybir.AluOpType.mult
ALU_ADD = mybir.AluOpType.add

_BUILT = None


# ======================= Launch 1: LN + QKV (row-parallel) ===================

def _emit_qkv(nc, tc, io):
    """QKV runs directly on raw x^T; LayerNorm is folded into the epilogue:
        W^T ((x - mu) * rstd) = (W^T x) * rstd[t] + nmr[t] * colsum(W)
    with nmr = -mu * rstd, so the normalize pass never blocks the PE."""
    xt_in = io["xt"].ap()        # [128, 8192]  x^T slice tiled [p, ko, t]
    w_qk = io["w_qk"].ap()       # [4096, 2048] tiled [fc*128+p, ko*128+f]
    w_v = io["w_v"].ap()         # [128, 32768] tiled [p, ko*2048+f]
    b_qk = io["b_qk"].ap()       # [128, 32] f32
    cs_qk = io["cs_qk"].ap()     # [128, 32] f32  colsum(w_qk) per-feature
    b_v = io["b_v"].ap()         # [2048] f32
    qk_out = io["qk"].ap()       # [4096, 512] bf16 out
    v_out = io["v"].ap()         # [2048, 512] bf16 out (tc*4+vc major)

    from contextlib import ExitStack

    with ExitStack() as es:
        constp = es.enter_context(tc.tile_pool(name="constp", bufs=1))
        persp = es.enter_context(tc.tile_pool(name="persp", bufs=1))
        ones_sb = constp.tile([P, 1], BF16, name="ones_sb")
        nc.any.memset(ones_sb[:], 1.0)
        eps_sb = constp.tile([1, 1], F32, name="eps_sb")
        nc.any.memset(eps_sb[:], EPS)

        # x^T slice first on the DMA queues — everything waits on it
        xt_sb = persp.tile([P, KC, RPC], BF16, name="xt_sb")
        for g in range(8):
            nc.sync.dma_start(xt_sb[:, 2 * g:2 * g + 2, :],
                              xt_in[:, g * 1024:(g + 1) * 1024])
        b_qk_sb = constp.tile([P, 32], F32, name="b_qk_sb")
        nc.sync.dma_start(b_qk_sb[:], b_qk)
        cs_qk_sb = constp.tile([P, 32], F32, name="cs_qk_sb")
        nc.sync.dma_start(cs_qk_sb[:], cs_qk)
        # w_v / b_v tiles are declared here but their loads are emitted after
        # the qk loop so they queue behind the w_qk stream (global DMA FIFO).
        b_v_sb = constp.tile([P, C], F32, name="b_v_sb")
        w_v_sb = constp.tile([P, KC, C], BF16, name="w_v_sb")

        # ---- LN stats via ones-matmuls (reduce over partition dim) ----
        lnw = es.enter_context(tc.tile_pool(name="lnw", bufs=4))
        lns = es.enter_context(tc.tile_pool(name="lns", bufs=1))
        lnps = es.enter_context(tc.tile_pool(name="lnps", bufs=1,
                                             space="PSUM"))
        sum_ps = lnps.tile([1, RPC], F32, name="sum_ps")
        sq_ps = lnps.tile([1, RPC], F32, name="sq_ps")
        for ko in range(KC):
            xsq = lnw.tile([P, RPC], BF16, tag="xsq")
            nc.vector.tensor_mul(xsq[:], xt_sb[:, ko, :], xt_sb[:, ko, :])
            nc.tensor.matmul(sum_ps[:], ones_sb[:], xt_sb[:, ko, :],
                             start=(ko == 0), stop=(ko == KC - 1))
            nc.tensor.matmul(sq_ps[:], ones_sb[:], xsq[:],
                             start=(ko == 0), stop=(ko == KC - 1))
        mu = lns.tile([1, RPC], F32, name="mu")
        nc.vector.tensor_scalar_mul(mu[:], sum_ps[:], 1.0 / C)
        ex2 = lns.tile([1, RPC], F32, name="ex2")
        nc.vector.tensor_scalar_mul(ex2[:], sq_ps[:], 1.0 / C)
        var = lns.tile([1, RPC], F32, name="var")
        nc.vector.tensor_mul(var[:], mu[:], mu[:])
        nc.vector.tensor_sub(var[:], ex2[:], var[:])
        std = lns.tile([1, RPC], F32, name="std")
        nc.scalar.activation(std[:], var[:], ACT.Sqrt, bias=eps_sb[:])
        rstd = lns.tile([1, RPC], F32, name="rstd")
        nc.vector.reciprocal(rstd[:], std[:])
        nmr = lns.tile([1, RPC], F32, name="nmr")
        nc.vector.tensor_mul(nmr[:], mu[:], rstd[:])
        nc.vector.tensor_scalar_mul(nmr[:], nmr[:], -1.0)
        # broadcast along partitions (for the feature-major qk epilogue)
        rstd_bc = lns.tile([P, RPC], F32, name="rstd_bc")
        nc.gpsimd.partition_broadcast(rstd_bc[:], rstd[:])
        nmr_bc = lns.tile([P, RPC], F32, name="nmr_bc")
        nc.gpsimd.partition_broadcast(nmr_bc[:], nmr[:])
        # normalized h tiles for the token-major v matmuls — produced on DVE
        # fully overlapped with the qk matmul stream (no PE bubble).
        h_sb = persp.tile([P, KC, RPC], BF16, name="h_sb")
        for ko in range(KC):
            tmp = lnw.tile([P, RPC], BF16, tag="xr")
            nc.vector.tensor_mul(tmp[:], xt_sb[:, ko, :], rstd_bc[:])
            nc.vector.tensor_add(h_sb[:, ko, :], tmp[:], nmr_bc[:])

        # ---- q^T / k^T: feature-major [fc*128, 512] ----
        with tc.tile_pool(name="wqkp", bufs=4) as wqkp, \
             tc.tile_pool(name="qkop", bufs=4) as qkop, \
             tc.tile_pool(name="qkps", bufs=3, space="PSUM") as qkps, \
             tc.tile_pool(name="vop", bufs=4) as vop, \
             tc.tile_pool(name="vps", bufs=3, space="PSUM") as vps:
            for fc in range(32):
                wt = wqkp.tile([P, KC, P], BF16, tag="wqk")
                nc.sync.dma_start(wt[:], w_qk[fc * P:(fc + 1) * P, :])
                ps = qkps.tile([P, RPC], F32, tag="qkps")
                for ko in range(KC):
                    nc.tensor.matmul(ps[:], wt[:, ko, :], xt_sb[:, ko, :],
                                     start=(ko == 0), stop=(ko == KC - 1))
                tmp = qkop.tile([P, RPC], F32, tag="qkt")
                nc.vector.tensor_mul(tmp[:], ps[:], rstd_bc[:])
                nc.vector.scalar_tensor_tensor(
                    tmp[:], nmr_bc[:], cs_qk_sb[:, fc:fc + 1], tmp[:],
                    ALU_MULT, ALU_ADD)
                ot = qkop.tile([P, RPC], BF16, tag="qko")
                nc.scalar.activation(ot[:], tmp[:], ACT.Identity,
                                     bias=b_qk_sb[:, fc:fc + 1], scale=1.0)
                nc.sync.dma_start(qk_out[fc * P:(fc + 1) * P, :], ot[:])

            for g in range(4):
                nc.sync.dma_start(w_v_sb[:, 4 * g:4 * g + 4, :],
                                  w_v[:, g * 8192:(g + 1) * 8192])
            nc.sync.dma_start(b_v_sb[:], b_v[None, :].to_broadcast((P, C)))

            # ---- v: token-major [tc*128, vc*512] ----
            for tc4 in range(4):
                for vc in range(4):
                    ps = vps.tile([P, RPC], F32, tag="vps")
                    for ko in range(KC):
                        nc.tensor.matmul(
                            ps[:], h_sb[:, ko, tc4 * P:(tc4 + 1) * P],
                            w_v_sb[:, ko, vc * 512:(vc + 1) * 512],
                            start=(ko == 0), stop=(ko == KC - 1))
                    ot = vop.tile([P, RPC], BF16, tag="vo")
                    nc.vector.tensor_add(ot[:], ps[:],
                                         b_v_sb[:, vc * 512:(vc + 1) * 512])
                    nc.sync.dma_start(
                        v_out[(tc4 * 4 + vc) * P:(tc4 * 4 + vc + 1) * P, :],
                        ot[:])


def _build_qkv():
    nc = bacc.Bacc("TRN2", target_bir_lowering=False, debug=False,
                   num_devices=N_CORES)
    io = {}
    io["xt"] = nc.dram_tensor("xt", [P, KC * RPC], BF16, kind="ExternalInput")
    io["w_qk"] = nc.dram_tensor("w_qk", [2 * C, C], BF16,
                                kind="ExternalInput")
    io["w_v"] = nc.dram_tensor("w_v", [P, KC * C], BF16, kind="ExternalInput")
    io["b_qk"] = nc.dram_tensor("b_qk", [P, 32], F32, kind="ExternalInput")
    io["cs_qk"] = nc.dram_tensor("cs_qk", [P, 32], F32, kind="ExternalInput")
    io["b_v"] = nc.dram_tensor("b_v", [C], F32, kind="ExternalInput")
    io["qk"] = nc.dram_tensor("qk", [2 * C, RPC], BF16, kind="ExternalOutput")
    io["v"] = nc.dram_tensor("v", [C, RPC], BF16, kind="ExternalOutput")
    with tile.TileContext(nc) as tc:
        _emit_qkv(nc, tc, io)
    nc.compile()
    return nc


# ======================= Launch 2: causal attention ==========================

def _emit_attn(nc, tc, io):
    qk4 = io["qk4"].ap()         # [128, 4*4096] bf16: [p, {q0,q1,k0,k1}, t]
    v32 = io["v32"].ap()         # [128, 32*256] bf16: [p, chunk, hl*128+d]
    cmask = io["cmask"].ap()     # [128, 128] bf16
    yt_out = io["yt"].ap()       # [256, 4096] bf16 out

    from contextlib import ExitStack

    with ExitStack() as es:
        constp = es.enter_context(tc.tile_pool(name="constp", bufs=1))
        persp = es.enter_context(tc.tile_pool(name="persp", bufs=1))
        ones_sb = constp.tile([P, 1], BF16, name="ones_sb")
        nc.any.memset(ones_sb[:], 1.0)

        qk_t = persp.tile([P, 4, TOK], BF16, name="qk_t")
        v_sb = persp.tile([P, NT, HPC * HD], BF16, name="v_sb")
        # need-ordered input streaming: batch 0's q0/k0 first so the first
        # S matmul can start after ~1MB, then v(b0), then the rest.
        NTB = T // P  # 16 v chunks per batch

        def load_qk(fc, b):
            nc.sync.dma_start(qk_t[:, fc, b * T:(b + 1) * T],
                              qk4[:, fc * TOK + b * T:fc * TOK + (b + 1) * T])

        def load_v(b, half):
            c0 = b * NTB + half * 8
            nc.sync.dma_start(v_sb[:, c0:c0 + 8, :],
                              v32[:, c0 * 256:(c0 + 8) * 256])

        load_qk(0, 0)
        load_qk(2, 0)
        load_v(0, 0)
        load_v(0, 1)
        load_qk(1, 0)
        load_qk(3, 0)
        load_qk(0, 1)
        load_qk(2, 1)
        load_v(1, 0)
        load_v(1, 1)
        load_qk(1, 1)
        load_qk(3, 1)
        mask_sb = constp.tile([P, P], BF16, name="mask_sb")
        nc.sync.dma_start(mask_sb[:], cmask)

        with tc.tile_pool(name="sps", bufs=3, space="PSUM") as sps, \
             tc.tile_pool(name="yps", bufs=2, space="PSUM") as yps, \
             tc.tile_pool(name="rps", bufs=2, space="PSUM") as rps, \
             tc.tile_pool(name="ptp", bufs=3) as ptp, \
             tc.tile_pool(name="attp", bufs=3) as attp:
            inv_sqrt_hd = 1.0 / math.sqrt(HD)
            for b in range(B):
                for hl in range(HPC):
                    q_sl = qk_t[:, hl, b * T:(b + 1) * T]
                    k_sl = qk_t[:, 2 + hl, b * T:(b + 1) * T]
                    for jt in range(JTT):
                        nblk = 4 * (jt + 1)
                        pt = ptp.tile([P, 16, 512], BF16, tag="pt")
                        y_ps = yps.tile([P, 512], F32, tag="y_ps")
                        rs_ps = rps.tile([1, 512], F32, tag="rs_ps")

                        def s_exp(ib):
                            c0 = max(0, ib * P - jt * 512)
                            s_ps = sps.tile([P, 512], F32, tag="s_ps")
                            nc.tensor.matmul(
                                s_ps[:, c0:512],
                                k_sl[:, ib * P:(ib + 1) * P],
                                q_sl[:, jt * 512 + c0:(jt + 1) * 512],
                                start=True, stop=True)
                            nc.scalar.activation(
                                pt[:, ib, c0:512], s_ps[:, c0:512],
                                ACT.Exp, scale=inv_sqrt_hd)
                            if ib >= 4 * jt:  # diagonal 128x128 sub-block
                                nc.vector.tensor_mul(
                                    pt[:, ib, c0:c0 + P],
                                    pt[:, ib, c0:c0 + P], mask_sb[:])

                        # software-pipelined by 2: the PE never sits at a
                        # y(ib) queue head waiting for exp(ib) — two more
                        # independent S matmuls are always in flight.
                        s_exp(0)
                        if nblk > 1:
                            s_exp(1)
                        for ib in range(nblk):
                            if ib + 2 < nblk:
                                s_exp(ib + 2)
                            c0 = max(0, ib * P - jt * 512)
                            vv = v_sb[:, b * NTB + ib, hl * HD:(hl + 1) * HD]
                            nc.tensor.matmul(
                                y_ps[:, c0:512], vv, pt[:, ib, c0:512],
                                start=(ib == 0), stop=(ib == nblk - 1))
                            nc.tensor.matmul(
                                rs_ps[:, c0:512], ones_sb[:],
                                pt[:, ib, c0:512],
                                start=(ib == 0), stop=(ib == nblk - 1))
                        rsv = attp.tile([1, 512], F32, tag="rsv")
                        nc.vector.reciprocal(rsv[:], rs_ps[:])
                        rbc = attp.tile([P, 512], F32, tag="rbc")
                        nc.gpsimd.partition_broadcast(rbc[:], rsv[:])
                        y_bf = attp.tile([P, 512], BF16, tag="y_bf")
                        nc.vector.tensor_mul(y_bf[:], y_ps[:], rbc[:])
                        nc.sync.dma_start(
                            yt_out[hl * HD:(hl + 1) * HD,
                                   b * T + jt * 512:b * T + (jt + 1) * 512],
                            y_bf[:])


def _build_attn():
    nc = bacc.Bacc("TRN2", target_bir_lowering=False, debug=False,
                   num_devices=N_CORES)
    io = {}
    io["qk4"] = nc.dram_tensor("qk4", [P, 4 * TOK], BF16,
                               kind="ExternalInput")
    io["v32"] = nc.dram_tensor("v32", [P, NT * HPC * HD], BF16,
                               kind="ExternalInput")
    io["cmask"] = nc.dram_tensor("cmask", [P, P], BF16, kind="ExternalInput")
    io["yt"] = nc.dram_tensor("yt", [HPC * HD, TOK], BF16,
                              kind="ExternalOutput")
    with tile.TileContext(nc) as tc:
        _emit_attn(nc, tc, io)
    nc.compile()
    return nc


# ======================= Launch 3: proj + MLP (row-parallel) =================

def _emit_mlp(nc, tc, io):
    yt_in = io["yt"].ap()        # [128, 8192]  Y^T slice tiled [p, ko, t]
    xt_in = io["xt"].ap()        # [128, 8192]  x^T slice tiled [p, ko, t]
    w_pr = io["w_pr"].ap()       # [2048, 2048] tiled [oc*128+p, ko*128+f]
    b_pr = io["b_pr"].ap()       # [128, 16] f32
    w_fc = io["w_fc"].ap()       # [8192, 2048] tiled [mc*128+p, ko*128+f]
    b_fc = io["b_fc"].ap()       # [128, 64] f32
    w_out = io["w_out"].ap()     # [4096, 4096] tiled [(ct*8+g)*128+p, m*512+f]
    b_out = io["b_out"].ap()     # [2048] f32
    ident = io["ident"].ap()     # [128, 128] bf16
    out = io["out"].ap()         # [512, 2048] f32 out

    from contextlib import ExitStack

    with ExitStack() as es:
        constp = es.enter_context(tc.tile_pool(name="constp", bufs=1))
        persp = es.enter_context(tc.tile_pool(name="persp", bufs=1))
        yt_sb = persp.tile([P, KC, RPC], BF16, name="yt_sb")
        xt_sb = persp.tile([P, KC, RPC], BF16, name="xt_sb")
        x2t_sb = persp.tile([P, KC, RPC], BF16, name="x2t_sb")
        m_sb = persp.tile([P, MC4, RPC], BF16, name="m_sb")
        b_pr_sb = constp.tile([P, 16], F32, name="b_pr_sb")
        nc.sync.dma_start(b_pr_sb[:], b_pr)

        # ---- proj + residual #1 (feature-major) ----
        # w_proj held whole; its chunk loads interleave with yt/xt so the
        # first matmul isn't stuck behind the bulk input stream (DMA
        # descriptors drain roughly globally-FIFO across the queues).
        with tc.tile_pool(name="wprp", bufs=1) as wprp, \
             tc.tile_pool(name="x2w", bufs=3) as x2w, \
             tc.tile_pool(name="pps", bufs=3, space="PSUM") as pps:
            w_pr_sb = wprp.tile([P, KC, KC, P], BF16, name="w_pr_sb")

            def load_wpr(c):
                nc.sync.dma_start(
                    w_pr_sb[:, 4 * c:4 * (c + 1), :, :],
                    w_pr[c * 4 * P:(c + 1) * 4 * P, :]
                    .rearrange("(o p) f -> p o f", p=P))

            load_wpr(0)
            nc.sync.dma_start(yt_sb[:, 0:4, :], yt_in[:, :2048])
            load_wpr(1)
            nc.sync.dma_start(yt_sb[:, 4:8, :], yt_in[:, 2048:4096])
            nc.sync.dma_start(xt_sb[:, 0:4, :], xt_in[:, :2048])
            load_wpr(2)
            nc.sync.dma_start(yt_sb[:, 8:12, :], yt_in[:, 4096:6144])
            load_wpr(3)
            nc.sync.dma_start(yt_sb[:, 12:16, :], yt_in[:, 6144:8192])
            for g in range(1, 4):
                nc.sync.dma_start(xt_sb[:, 4 * g:4 * g + 4, :],
                                  xt_in[:, g * 2048:(g + 1) * 2048])
            b_fc_sb = constp.tile([P, MC4], F32, name="b_fc_sb")
            nc.sync.dma_start(b_fc_sb[:], b_fc)
            i_sb = constp.tile([P, P], BF16, name="i_sb")
            nc.sync.dma_start(i_sb[:], ident)
            b_out_sb = constp.tile([P, C], F32, name="b_out_sb")

            for oc in range(KC):
                ps = pps.tile([P, RPC], F32, tag="pps")
                for ko in range(KC):
                    nc.tensor.matmul(ps[:], w_pr_sb[:, oc, ko, :],
                                     yt_sb[:, ko, :],
                                     start=(ko == 0), stop=(ko == KC - 1))
                tmp = x2w.tile([P, RPC], F32, tag="x2f")
                nc.vector.tensor_add(tmp[:], ps[:], xt_sb[:, oc, :])
                nc.scalar.activation(x2t_sb[:, oc, :], tmp[:], ACT.Identity,
                                     bias=b_pr_sb[:, oc:oc + 1], scale=1.0)

        # ---- fc + gelu (feature-major) ----
        with tc.tile_pool(name="wfcp", bufs=4) as wfcp, \
             tc.tile_pool(name="wop", bufs=4) as wop, \
             tc.tile_pool(name="fps", bufs=3, space="PSUM") as fps:
            for mc in range(MC4):
                wt = wfcp.tile([P, KC, P], BF16, tag="wfc")
                nc.sync.dma_start(wt[:], w_fc[mc * P:(mc + 1) * P, :])
                ps = fps.tile([P, RPC], F32, tag="fps")
                for ko in range(KC):
                    nc.tensor.matmul(ps[:], wt[:, ko, :], x2t_sb[:, ko, :],
                                     start=(ko == 0), stop=(ko == KC - 1))
                nc.scalar.activation(m_sb[:, mc, :], ps[:], ACT.Gelu,
                                     bias=b_fc_sb[:, mc:mc + 1], scale=1.0)
                if mc == 0:
                    # prefetch the first w_out group tiles + b_out now so the
                    # fc->out transition never waits on the DMA FIFO
                    nc.sync.dma_start(
                        b_out_sb[:], b_out[None, :].to_broadcast((P, C)))
                    wo_pre = []
                    for g in range(2):
                        wt0 = wop.tile([P, 8, 512], BF16, tag="wo",
                                       name=f"wo_pre{g}")
                        nc.sync.dma_start(wt0[:], w_out[g * P:(g + 1) * P, :])
                        wo_pre.append(wt0)

            # ---- out matmul + residual #2 (token-major; x2 via identity) --
            with tc.tile_pool(name="ofp", bufs=3) as ofp, \
                 tc.tile_pool(name="ops", bufs=1, space="PSUM") as ops:
                for ct in range(4):
                    pss = [ops.tile([P, 512], F32, tag=f"o_ps{rb}",
                                    name=f"o_ps{rb}_{ct}")
                           for rb in range(4)]
                    for g in range(8):
                        if ct == 0 and g < 2:
                            wt = wo_pre[g]
                        else:
                            wt = wop.tile([P, 8, 512], BF16, tag="wo")
                            nc.sync.dma_start(
                                wt[:],
                                w_out[(ct * 8 + g) * P:(ct * 8 + g + 1) * P, :])
                        for rb in range(4):
                            for m8 in range(8):
                                nc.tensor.matmul(
                                    pss[rb][:],
                                    m_sb[:, g * 8 + m8, rb * P:(rb + 1) * P],
                                    wt[:, m8, :],
                                    start=(g == 0 and m8 == 0), stop=False)
                    for rb in range(4):
                        for cc in range(4):
                            nc.tensor.matmul(
                                pss[rb][:, cc * P:(cc + 1) * P],
                                x2t_sb[:, ct * 4 + cc, rb * P:(rb + 1) * P],
                                i_sb[:],
                                start=False, stop=(cc == 3),
                                skip_group_check=True)
                        of = ofp.tile([P, 512], F32, tag="of")
                        nc.vector.tensor_add(
                            of[:], pss[rb][:],
                            b_out_sb[:, ct * 512:(ct + 1) * 512])
                        nc.sync.dma_start(
                            out[rb * P:(rb + 1) * P, ct * 512:(ct + 1) * 512],
                            of[:])


def _build_mlp():
    nc = bacc.Bacc("TRN2", target_bir_lowering=False, debug=False,
                   num_devices=N_CORES)
    io = {}
    io["yt"] = nc.dram_tensor("yt", [P, KC * RPC], BF16, kind="ExternalInput")
    io["xt"] = nc.dram_tensor("xt", [P, KC * RPC], BF16, kind="ExternalInput")
    io["w_pr"] = nc.dram_tensor("w_pr", [C, C], BF16, kind="ExternalInput")
    io["b_pr"] = nc.dram_tensor("b_pr", [P, KC], F32, kind="ExternalInput")
    io["w_fc"] = nc.dram_tensor("w_fc", [F4, C], BF16, kind="ExternalInput")
    io["b_fc"] = nc.dram_tensor("b_fc", [P, MC4], F32, kind="ExternalInput")
    io["w_out"] = nc.dram_tensor("w_out", [2 * C, 2 * C], BF16,
                                 kind="ExternalInput")
    io["b_out"] = nc.dram_tensor("b_out", [C], F32, kind="ExternalInput")
    io["ident"] = nc.dram_tensor("ident", [P, P], BF16, kind="ExternalInput")
    io["out"] = nc.dram_tensor("out", [RPC, C], F32, kind="ExternalOutput")
    with tile.TileContext(nc) as tc:
        _emit_mlp(nc, tc, io)
    nc.compile()
    return nc


def _get_built():
    global _BUILT
    if _BUILT is None:
        _BUILT = (_build_qkv(), _build_attn(), _build_mlp())
    return _BUILT


# ======================= Host orchestration ==================================

def _tile_km(w, nchunk):
    """[K, M] -> [M/128 * 128, K/128 * 128] tiled rows: out[fc*128+p, ko*128+f]
    = w[ko*128+p, fc*128+f]."""
    K, M = w.shape
    return np.ascontiguousarray(
        w.reshape(K // P, P, nchunk, M // nchunk)
        .transpose(2, 1, 0, 3)
        .reshape(nchunk * P, K // P * (M // nchunk)))


def _tile_xt(xt_slice):
    """[2048, 512] feature-major slice -> [128, 16*512] tiled [p, ko, t]."""
    return np.ascontiguousarray(
        xt_slice.reshape(KC, P, RPC).transpose(1, 0, 2).reshape(P, KC * RPC))


def _prep(x, ln_scale, ln_bias, w_qkv, b_qkv, w_proj, b_proj,
          w_fc, b_fc, w_out, b_out):
    bf = ml_dtypes.bfloat16
    xf = np.asarray(x, np.float32).reshape(TOK, C)
    xT_bf = np.ascontiguousarray(xf.T).astype(bf)        # [C, TOK]

    # Fold LN affine into the QKV projection (exact, in float64).
    w64 = np.asarray(w_qkv, np.float64)
    w_eff = np.asarray(ln_scale, np.float64)[:, None] * w64
    b_eff = np.asarray(b_qkv, np.float64) + np.asarray(ln_bias,
                                                       np.float64) @ w64
    wqk = np.concatenate([w_eff[:, :C], w_eff[:, C:2 * C]],
                         axis=1).astype(np.float32).astype(bf)   # [C, 2C]
    wv = w_eff[:, 2 * C:].astype(np.float32).astype(bf)          # [C, C]
    bqk = np.concatenate([b_eff[:C], b_eff[C:2 * C]]).astype(np.float32)
    bv = np.ascontiguousarray(b_eff[2 * C:].astype(np.float32))

    w_qk_t = _tile_km(wqk, 32)                                   # [4096, 2048]
    w_v_t = np.ascontiguousarray(
        wv.reshape(KC, P, C).transpose(1, 0, 2).reshape(P, KC * C))
    b_qk_t = np.ascontiguousarray(bqk.reshape(32, P).T)          # [128, 32]
    cs_qk_t = np.ascontiguousarray(
        wqk.astype(np.float32).sum(0).reshape(32, P).T)          # [128, 32]

    xts = [_tile_xt(xT_bf[:, i * RPC:(i + 1) * RPC]) for i in range(N_CORES)]

    in1 = [{"xt": xts[i], "w_qk": w_qk_t, "w_v": w_v_t,
            "b_qk": b_qk_t, "cs_qk": cs_qk_t, "b_v": bv}
           for i in range(N_CORES)]

    w_pr_t = _tile_km(np.asarray(w_proj, np.float32).astype(bf), KC)
    b_pr_t = np.ascontiguousarray(
        np.asarray(b_proj, np.float32).reshape(KC, P).T)
    w_fc_t = _tile_km(np.asarray(w_fc, np.float32).astype(bf), MC4)
    b_fc_t = np.ascontiguousarray(
        np.asarray(b_fc, np.float32).reshape(MC4, P).T)
    # w_out [8192, 2048] -> [32*128, 4096]: [ct*8+g][p][m*512+f]
    w_out_t = np.ascontiguousarray(
        np.asarray(w_out, np.float32).astype(bf)
        .reshape(8, 8, P, 4, 512).transpose(3, 0, 2, 1, 4)
        .reshape(32 * P, 4096))
    b_out_f = np.ascontiguousarray(np.asarray(b_out, np.float32))
    ident = np.eye(P, dtype=np.float32).astype(bf)
    cmask = np.triu(np.ones((P, P), np.float32)).astype(bf)
    in3_common = {"w_pr": w_pr_t, "b_pr": b_pr_t, "w_fc": w_fc_t,
                  "b_fc": b_fc_t, "w_out": w_out_t, "b_out": b_out_f,
                  "ident": ident}
    return xts, in1, in3_common, cmask


_LAUNCHERS = None  # test hook: list of 3 callables (in_maps, kwargs) -> res


class _EmuRes:
    def __init__(self, results):
        self.results = results
        self.exec_time_ns = None
        self.instructions_and_trace = None


def run(inputs, trace=False, trace_cores=None):
    """Run the three SPMD launches. Returns (out [B,T,C] f32, [res1,res2,res3])."""
    xts, in1, in3_common, cmask = _prep(**inputs)
    kwargs = {}
    if trace:
        kwargs = dict(trace=True,
                      trace_cores=trace_cores if trace_cores else [0])
    cores = list(range(N_CORES))

    if _LAUNCHERS is None:
        nc1, nc2, nc3 = _get_built()
        launch = [
            lambda ims, kw: run_bass_kernel_spmd(nc1, ims, core_ids=cores, **kw),
            lambda ims, kw: run_bass_kernel_spmd(nc2, ims, core_ids=cores, **kw),
            lambda ims, kw: run_bass_kernel_spmd(nc3, ims, core_ids=cores, **kw),
        ]
    else:
        launch = _LAUNCHERS

    res1 = launch[0](in1, kwargs)
    # qk [4096, 512] per core -> qkT_full [4096, 4096]
    qkT_full = np.concatenate(
        [np.asarray(res1.results[i]["qk"]) for i in range(N_CORES)], axis=1)
    # v [2048, 512] per core: [(tc*4+vc)*128+p, f] -> v_full [4096, 2048]
    v_full = np.concatenate(
        [np.asarray(res1.results[i]["v"])
         .reshape(4, 4, P, 512).transpose(0, 2, 1, 3).reshape(RPC, C)
         for i in range(N_CORES)], axis=0)

    qT, kT = qkT_full[:C], qkT_full[C:]
    v_r = v_full.reshape(NT, P, H, HD)
    in2 = []
    for c in range(N_CORES):
        h0 = HPC * c
        qk4 = np.ascontiguousarray(np.stack(
            [qT[h0 * HD:(h0 + 1) * HD], qT[(h0 + 1) * HD:(h0 + 2) * HD],
             kT[h0 * HD:(h0 + 1) * HD], kT[(h0 + 1) * HD:(h0 + 2) * HD]],
            axis=1).reshape(P, 4 * TOK))
        v32 = np.ascontiguousarray(
            v_r[:, :, h0:h0 + HPC, :].transpose(1, 0, 2, 3)
            .reshape(P, NT * HPC * HD))
        in2.append({"qk4": qk4, "v32": v32, "cmask": cmask})
    res2 = launch[1](in2, kwargs)
    yT_full = np.concatenate(
        [np.asarray(res2.results[i]["yt"]) for i in range(N_CORES)], axis=0)

    in3 = []
    for i in range(N_CORES):
        in3.append({"yt": _tile_xt(yT_full[:, i * RPC:(i + 1) * RPC]),
                    "xt": xts[i], **in3_common})
    res3 = launch[2](in3, kwargs)
    outf = np.concatenate(
        [np.asarray(res3.results[i]["out"]) for i in range(N_CORES)], axis=0)
    return outf.reshape(B, T, C).astype(np.float32), [res1, res2, res3]


def kernel(**inputs):
    out, _ = run(inputs, trace=False)
    return out


# revision 37
# speedup vs baseline: 1.0095x; 1.0095x over previous
"""Trainium2 Bass kernel for nn_Block_24111946399747 (dense transformer block).

Three balanced SPMD launches (host resharding between launches; no on-device
collectives available on this terminal):

L1 (row-parallel; core i owns tokens 512i..512i+512):
  - LayerNorm in feature-major layout from a host-pretransposed x^T slice
    (LN affine folded into the QKV weights on host).  Stats (sum, sum-sq)
    via ones-matmuls over the partition dim; normalize on DVE.  Zero
    transposes anywhere.
  - QKV: q^T,k^T feature-major [4096f, 512t] (lhsT = W chunks) and v
    token-major [512t, 2048f] (lhsT = h^T chunks).

L2 (head-parallel attention; core i owns heads 2i, 2i+1; causal work is
  identical per core):
      S^T  [tk, tq] = k_t.T @ q_t ;  P^T = exp(S^T/sqrt(HD)) (causal)
      y_t  [HD, tq] = v^T @ P^T    (rowsum via ones-matmul, then scale)
  q/k/v arrive pre-laid-out from the host; returns y_t [256, 4096] bf16.

L3 (row-parallel proj+MLP; core i owns tokens 512i..512i+512):
  - x2^T = w_proj^T @ Y^T + x^T + b  (feature-major, rhs = Y^T)
  - m^T  = gelu(w_fc^T @ x2^T + b)   (feature-major)
  - out  = m @ w_out + x2 + b        (token-major; lhsT = m^T slices; the
    x2 residual is folded in as identity matmuls, so no transpose is needed)

Matmuls in bf16 with fp32 PSUM accumulation; LN stats, softmax sums in fp32.
"""

import math
import sys

import numpy as np

if "/opt/trn_rl_repo" not in sys.path:
    sys.path.insert(0, "/opt/trn_rl_repo")

import ml_dtypes  # noqa: E402

import concourse.bass as bass  # noqa: E402,F401
import concourse.mybir as mybir  # noqa: E402
import concourse.tile as tile  # noqa: E402
from concourse import bacc  # noqa: E402
from concourse.bass_utils import run_bass_kernel_spmd  # noqa: E402

B, T, C, H = 2, 2048, 2048, 16
HD = C // H            # 128 head dim
N_CORES = 8
HPC = H // N_CORES     # 2 heads per core
TOK = B * T            # 4096 tokens
RPC = TOK // N_CORES   # 512 rows (tokens) per core
P = 128
KC = C // P            # 16 contraction chunks over C
F4 = 4 * C             # 8192
MC4 = F4 // P          # 64 feature chunks over 4C
NT = TOK // P          # 32 token tiles of 128
JTT = T // 512         # 4 query tiles of 512 per batch
EPS = 1e-6
BF16 = mybir.dt.bfloat16
F32 = mybir.dt.float32
ACT = mybir.ActivationFunctionType
ALU_MULT = mybir.AluOpType.mult
ALU_ADD = mybir.AluOpType.add

_BUILT = None


# ======================= Launch 1: LN + QKV (row-parallel) ===================

def _emit_qkv(nc, tc, io):
    """QKV runs directly on raw x^T; LayerNorm is folded into the epilogue:
        W^T ((x - mu) * rstd) = (W^T x) * rstd[t] + nmr[t] * colsum(W)
    with nmr = -mu * rstd, so the normalize pass never blocks the PE."""
    xt_in = io["xt"].ap()        # [128, 8192]  x^T slice tiled [p, ko, t]
    w_qk = io["w_qk"].ap()       # [4096, 2048] tiled [fc*128+p, ko*128+f]
    w_v = io["w_v"].ap()         # [128, 32768] tiled [p, ko*2048+f]
    b_qk = io["b_qk"].ap()       # [128, 32] f32
    cs_qk = io["cs_qk"].ap()     # [128, 32] f32  colsum(w_qk) per-feature
    b_v = io["b_v"].ap()         # [2048] f32
    qk_out = io["qk"].ap()       # [4096, 512] bf16 out
    v_out = io["v"].ap()         # [2048, 512] bf16 out (tc*4+vc major)

    from contextlib import ExitStack

    with ExitStack() as es:
        constp = es.enter_context(tc.tile_pool(name="constp", bufs=1))
        persp = es.enter_context(tc.tile_pool(name="persp", bufs=1))
        ones_sb = constp.tile([P, 1], BF16, name="ones_sb")
        nc.any.memset(ones_sb[:], 1.0)
        eps_sb = constp.tile([1, 1], F32, name="eps_sb")
        nc.any.memset(eps_sb[:], EPS)

        # x^T slice first on the DMA queues — everything waits on it
        xt_sb = persp.tile([P, KC, RPC], BF16, name="xt_sb")
        for g in range(8):
            nc.sync.dma_start(xt_sb[:, 2 * g:2 * g + 2, :],
                              xt_in[:, g * 1024:(g + 1) * 1024])
        b_qk_sb = constp.tile([P, 32], F32, name="b_qk_sb")
        nc.sync.dma_start(b_qk_sb[:], b_qk)
        cs_qk_sb = constp.tile([P, 32], F32, name="cs_qk_sb")
        nc.sync.dma_start(cs_qk_sb[:], cs_qk)
        # w_v / b_v tiles are declared here but their loads are emitted after
        # the qk loop so they queue behind the w_qk stream (global DMA FIFO).
        b_v_sb = constp.tile([P, C], F32, name="b_v_sb")
        w_v_sb = constp.tile([P, KC, C], BF16, name="w_v_sb")

        # ---- LN stats via ones-matmuls (reduce over partition dim) ----
        lnw = es.enter_context(tc.tile_pool(name="lnw", bufs=4))
        lns = es.enter_context(tc.tile_pool(name="lns", bufs=1))
        lnps = es.enter_context(tc.tile_pool(name="lnps", bufs=1,
                                             space="PSUM"))
        sum_ps = lnps.tile([1, RPC], F32, name="sum_ps")
        sq_ps = lnps.tile([1, RPC], F32, name="sq_ps")
        for ko in range(KC):
            xsq = lnw.tile([P, RPC], BF16, tag="xsq")
            nc.vector.tensor_mul(xsq[:], xt_sb[:, ko, :], xt_sb[:, ko, :])
            nc.tensor.matmul(sum_ps[:], ones_sb[:], xt_sb[:, ko, :],
                             start=(ko == 0), stop=(ko == KC - 1))
            nc.tensor.matmul(sq_ps[:], ones_sb[:], xsq[:],
                             start=(ko == 0), stop=(ko == KC - 1))
        mu = lns.tile([1, RPC], F32, name="mu")
        nc.vector.tensor_scalar_mul(mu[:], sum_ps[:], 1.0 / C)
        ex2 = lns.tile([1, RPC], F32, name="ex2")
        nc.vector.tensor_scalar_mul(ex2[:], sq_ps[:], 1.0 / C)
        var = lns.tile([1, RPC], F32, name="var")
        nc.vector.tensor_mul(var[:], mu[:], mu[:])
        nc.vector.tensor_sub(var[:], ex2[:], var[:])
        std = lns.tile([1, RPC], F32, name="std")
        nc.scalar.activation(std[:], var[:], ACT.Sqrt, bias=eps_sb[:])
        rstd = lns.tile([1, RPC], F32, name="rstd")
        nc.vector.reciprocal(rstd[:], std[:])
        nmr = lns.tile([1, RPC], F32, name="nmr")
        nc.vector.tensor_mul(nmr[:], mu[:], rstd[:])
        nc.vector.tensor_scalar_mul(nmr[:], nmr[:], -1.0)
        # broadcast along partitions (for the feature-major qk epilogue)
        rstd_bc = lns.tile([P, RPC], F32, name="rstd_bc")
        nc.gpsimd.partition_broadcast(rstd_bc[:], rstd[:])
        nmr_bc = lns.tile([P, RPC], F32, name="nmr_bc")
        nc.gpsimd.partition_broadcast(nmr_bc[:], nmr[:])
        # normalized h tiles for the token-major v matmuls — produced on DVE
        # fully overlapped with the qk matmul stream (no PE bubble).
        h_sb = persp.tile([P, KC, RPC], BF16, name="h_sb")
        for ko in range(KC):
            tmp = lnw.tile([P, RPC], BF16, tag="xr")
            nc.vector.tensor_mul(tmp[:], xt_sb[:, ko, :], rstd_bc[:])
            nc.vector.tensor_add(h_sb[:, ko, :], tmp[:], nmr_bc[:])

        # ---- q^T / k^T: feature-major [fc*128, 512] ----
        with tc.tile_pool(name="wqkp", bufs=6) as wqkp, \
             tc.tile_pool(name="qkop", bufs=4) as qkop, \
             tc.tile_pool(name="qkps", bufs=3, space="PSUM") as qkps, \
             tc.tile_pool(name="vop", bufs=4) as vop, \
             tc.tile_pool(name="vps", bufs=3, space="PSUM") as vps:
            for fc in range(32):
                wt = wqkp.tile([P, KC, P], BF16, tag="wqk")
                nc.sync.dma_start(wt[:], w_qk[fc * P:(fc + 1) * P, :])
                ps = qkps.tile([P, RPC], F32, tag="qkps")
                for ko in range(KC):
                    nc.tensor.matmul(ps[:], wt[:, ko, :], xt_sb[:, ko, :],
                                     start=(ko == 0), stop=(ko == KC - 1))
                tmp = qkop.tile([P, RPC], F32, tag="qkt")
                nc.vector.tensor_mul(tmp[:], ps[:], rstd_bc[:])
                nc.vector.scalar_tensor_tensor(
                    tmp[:], nmr_bc[:], cs_qk_sb[:, fc:fc + 1], tmp[:],
                    ALU_MULT, ALU_ADD)
                ot = qkop.tile([P, RPC], BF16, tag="qko")
                nc.scalar.activation(ot[:], tmp[:], ACT.Identity,
                                     bias=b_qk_sb[:, fc:fc + 1], scale=1.0)
                nc.sync.dma_start(qk_out[fc * P:(fc + 1) * P, :], ot[:])

            for g in range(4):
                nc.sync.dma_start(w_v_sb[:, 4 * g:4 * g + 4, :],
                                  w_v[:, g * 8192:(g + 1) * 8192])
            nc.sync.dma_start(b_v_sb[:], b_v[None, :].to_broadcast((P, C)))

            # ---- v: token-major [tc*128, vc*512] ----
            for tc4 in range(4):
                for vc in range(4):
                    ps = vps.tile([P, RPC], F32, tag="vps")
                    for ko in range(KC):
                        nc.tensor.matmul(
                            ps[:], h_sb[:, ko, tc4 * P:(tc4 + 1) * P],
                            w_v_sb[:, ko, vc * 512:(vc + 1) * 512],
                            start=(ko == 0), stop=(ko == KC - 1))
                    ot = vop.tile([P, RPC], BF16, tag="vo")
                    nc.vector.tensor_add(ot[:], ps[:],
                                         b_v_sb[:, vc * 512:(vc + 1) * 512])
                    nc.sync.dma_start(
                        v_out[(tc4 * 4 + vc) * P:(tc4 * 4 + vc + 1) * P, :],
                        ot[:])


def _build_qkv():
    nc = bacc.Bacc("TRN2", target_bir_lowering=False, debug=False,
                   num_devices=N_CORES)
    io = {}
    io["xt"] = nc.dram_tensor("xt", [P, KC * RPC], BF16, kind="ExternalInput")
    io["w_qk"] = nc.dram_tensor("w_qk", [2 * C, C], BF16,
                                kind="ExternalInput")
    io["w_v"] = nc.dram_tensor("w_v", [P, KC * C], BF16, kind="ExternalInput")
    io["b_qk"] = nc.dram_tensor("b_qk", [P, 32], F32, kind="ExternalInput")
    io["cs_qk"] = nc.dram_tensor("cs_qk", [P, 32], F32, kind="ExternalInput")
    io["b_v"] = nc.dram_tensor("b_v", [C], F32, kind="ExternalInput")
    io["qk"] = nc.dram_tensor("qk", [2 * C, RPC], BF16, kind="ExternalOutput")
    io["v"] = nc.dram_tensor("v", [C, RPC], BF16, kind="ExternalOutput")
    with tile.TileContext(nc) as tc:
        _emit_qkv(nc, tc, io)
    nc.compile()
    return nc


# ======================= Launch 2: causal attention ==========================

def _emit_attn(nc, tc, io):
    qk4 = io["qk4"].ap()         # [128, 4*4096] bf16: [p, {q0,q1,k0,k1}, t]
    v32 = io["v32"].ap()         # [128, 32*256] bf16: [p, chunk, hl*128+d]
    cmask = io["cmask"].ap()     # [128, 128] bf16
    yt_out = io["yt"].ap()       # [256, 4096] bf16 out

    from contextlib import ExitStack

    with ExitStack() as es:
        constp = es.enter_context(tc.tile_pool(name="constp", bufs=1))
        persp = es.enter_context(tc.tile_pool(name="persp", bufs=1))
        ones_sb = constp.tile([P, 1], BF16, name="ones_sb")
        nc.any.memset(ones_sb[:], 1.0)

        qk_t = persp.tile([P, 4, TOK], BF16, name="qk_t")
        v_sb = persp.tile([P, NT, HPC * HD], BF16, name="v_sb")
        # need-ordered input streaming: batch 0's q0/k0 first so the first
        # S matmul can start after ~1MB, then v(b0), then the rest.
        NTB = T // P  # 16 v chunks per batch

        def load_qk(fc, b):
            nc.sync.dma_start(qk_t[:, fc, b * T:(b + 1) * T],
                              qk4[:, fc * TOK + b * T:fc * TOK + (b + 1) * T])

        def load_v(b, half):
            c0 = b * NTB + half * 8
            nc.sync.dma_start(v_sb[:, c0:c0 + 8, :],
                              v32[:, c0 * 256:(c0 + 8) * 256])

        load_qk(0, 0)
        load_qk(2, 0)
        load_v(0, 0)
        load_v(0, 1)
        load_qk(1, 0)
        load_qk(3, 0)
        load_qk(0, 1)
        load_qk(2, 1)
        load_v(1, 0)
        load_v(1, 1)
        load_qk(1, 1)
        load_qk(3, 1)
        mask_sb = constp.tile([P, P], BF16, name="mask_sb")
        nc.sync.dma_start(mask_sb[:], cmask)

        from collections import deque

        with tc.tile_pool(name="sps", bufs=4, space="PSUM") as sps, \
             tc.tile_pool(name="yps", bufs=2, space="PSUM") as yps, \
             tc.tile_pool(name="rps", bufs=2, space="PSUM") as rps, \
             tc.tile_pool(name="ptp", bufs=3) as ptp, \
             tc.tile_pool(name="attp", bufs=3) as attp:
            inv_sqrt_hd = 1.0 / math.sqrt(HD)
            # Software pipeline ACROSS (b, hl, jt) block boundaries: the
            # S-matmul+exp stream runs 2 tasks ahead of the y/rowsum stream,
            # so the PE never drains at a block tail waiting for the last exp.
            groups = [(b, hl, jt) for b in range(B) for hl in range(HPC)
                      for jt in range(JTT)]
            state = {}
            pend = deque()

            def s_exp(gi, ib):
                st = state[gi]
                jt = st["jt"]
                c0 = max(0, ib * P - jt * 512)
                s_ps = sps.tile([P, 512], F32, tag="s_ps")
                nc.tensor.matmul(
                    s_ps[:, c0:512],
                    st["k_sl"][:, ib * P:(ib + 1) * P],
                    st["q_sl"][:, jt * 512 + c0:(jt + 1) * 512],
                    start=True, stop=True)
                nc.scalar.activation(
                    st["pt"][:, ib, c0:512], s_ps[:, c0:512],
                    ACT.Exp, scale=inv_sqrt_hd)
                if ib >= 4 * jt:  # diagonal 128x128 sub-block
                    nc.vector.tensor_mul(
                        st["pt"][:, ib, c0:c0 + P],
                        st["pt"][:, ib, c0:c0 + P], mask_sb[:])

            def emit_y_rs(gi, ib):
                st = state[gi]
                b, hl, jt, nblk = st["b"], st["hl"], st["jt"], st["nblk"]
                c0 = max(0, ib * P - jt * 512)
                vv = v_sb[:, b * NTB + ib, hl * HD:(hl + 1) * HD]
                nc.tensor.matmul(
                    st["y_ps"][:, c0:512], vv, st["pt"][:, ib, c0:512],
                    start=(ib == 0), stop=(ib == nblk - 1))
                nc.tensor.matmul(
                    st["rs_ps"][:, c0:512], ones_sb[:], st["pt"][:, ib, c0:512],
                    start=(ib == 0), stop=(ib == nblk - 1))
                if ib == nblk - 1:
                    rsv = attp.tile([1, 512], F32, tag="rsv")
                    nc.vector.reciprocal(rsv[:], st["rs_ps"][:])
                    rbc = attp.tile([P, 512], F32, tag="rbc")
                    nc.gpsimd.partition_broadcast(rbc[:], rsv[:])
                    y_bf = attp.tile([P, 512], BF16, tag="y_bf")
                    nc.vector.tensor_mul(y_bf[:], st["y_ps"][:], rbc[:])
                    nc.sync.dma_start(
                        yt_out[hl * HD:(hl + 1) * HD,
                               b * T + jt * 512:b * T + (jt + 1) * 512],
                        y_bf[:])
                    del state[gi]

            for gi, (b, hl, jt) in enumerate(groups):
                nblk = 4 * (jt + 1)
                state[gi] = {
                    "b": b, "hl": hl, "jt": jt, "nblk": nblk,
                    "q_sl": qk_t[:, hl, b * T:(b + 1) * T],
                    "k_sl": qk_t[:, 2 + hl, b * T:(b + 1) * T],
                    "pt": ptp.tile([P, 16, 512], BF16, tag="pt",
                                   name=f"pt_{gi}"),
                    "y_ps": yps.tile([P, 512], F32, tag="y_ps",
                                     name=f"y_ps_{gi}"),
                    "rs_ps": rps.tile([1, 512], F32, tag="rs_ps",
                                      name=f"rs_ps_{gi}"),
                }
                for ib in range(nblk):
                    s_exp(gi, ib)
                    pend.append((gi, ib))
                    if len(pend) > 2:
                        emit_y_rs(*pend.popleft())
            while pend:
                emit_y_rs(*pend.popleft())


def _build_attn():
    nc = bacc.Bacc("TRN2", target_bir_lowering=False, debug=False,
                   num_devices=N_CORES)
    io = {}
    io["qk4"] = nc.dram_tensor("qk4", [P, 4 * TOK], BF16,
                               kind="ExternalInput")
    io["v32"] = nc.dram_tensor("v32", [P, NT * HPC * HD], BF16,
                               kind="ExternalInput")
    io["cmask"] = nc.dram_tensor("cmask", [P, P], BF16, kind="ExternalInput")
    io["yt"] = nc.dram_tensor("yt", [HPC * HD, TOK], BF16,
                              kind="ExternalOutput")
    with tile.TileContext(nc) as tc:
        _emit_attn(nc, tc, io)
    nc.compile()
    return nc


# ======================= Launch 3: proj + MLP (row-parallel) =================

def _emit_mlp(nc, tc, io):
    yt_in = io["yt"].ap()        # [128, 8192]  Y^T slice tiled [p, ko, t]
    xt_in = io["xt"].ap()        # [128, 8192]  x^T slice tiled [p, ko, t]
    w_pr = io["w_pr"].ap()       # [2048, 2048] tiled [oc*128+p, ko*128+f]
    w_fc = io["w_fc"].ap()       # [8192, 2048] tiled [mc*128+p, ko*128+f]
    b_fc = io["b_fc"].ap()       # [128, 64] f32
    w_out = io["w_out"].ap()     # [4096, 4096] tiled [(ct*8+g)*128+p, m*512+f]
    b_out = io["b_out"].ap()     # [2048] f32
    ident = io["ident"].ap()     # [128, 128] bf16
    out = io["out"].ap()         # [512, 2048] f32 out

    from contextlib import ExitStack

    with ExitStack() as es:
        constp = es.enter_context(tc.tile_pool(name="constp", bufs=1))
        persp = es.enter_context(tc.tile_pool(name="persp", bufs=1))
        yt_sb = persp.tile([P, KC, RPC], BF16, name="yt_sb")
        xt_sb = persp.tile([P, KC, RPC], BF16, name="xt_sb")
        x2t_sb = persp.tile([P, KC, RPC], BF16, name="x2t_sb")
        m_sb = persp.tile([P, MC4, RPC], BF16, name="m_sb")
        # ---- proj + residual #1 (feature-major) ----
        # (b_proj is pre-added into the x^T input on the host)
        # w_proj held whole; its chunk loads interleave with yt/xt so the
        # first matmul isn't stuck behind the bulk input stream (DMA
        # descriptors drain roughly globally-FIFO across the queues).
        with tc.tile_pool(name="wprp", bufs=1) as wprp, \
             tc.tile_pool(name="pps", bufs=3, space="PSUM") as pps:
            w_pr_sb = wprp.tile([P, KC, KC, P], BF16, name="w_pr_sb")

            def load_wpr(c):
                nc.sync.dma_start(
                    w_pr_sb[:, 4 * c:4 * (c + 1), :, :],
                    w_pr[c * 4 * P:(c + 1) * 4 * P, :]
                    .rearrange("(o p) f -> p o f", p=P))

            load_wpr(0)
            nc.sync.dma_start(yt_sb[:, 0:4, :], yt_in[:, :2048])
            load_wpr(1)
            nc.sync.dma_start(yt_sb[:, 4:8, :], yt_in[:, 2048:4096])
            nc.sync.dma_start(xt_sb[:, 0:4, :], xt_in[:, :2048])
            load_wpr(2)
            nc.sync.dma_start(yt_sb[:, 8:12, :], yt_in[:, 4096:6144])
            load_wpr(3)
            nc.sync.dma_start(yt_sb[:, 12:16, :], yt_in[:, 6144:8192])
            for g in range(1, 4):
                nc.sync.dma_start(xt_sb[:, 4 * g:4 * g + 4, :],
                                  xt_in[:, g * 2048:(g + 1) * 2048])
            b_fc_sb = constp.tile([P, MC4], F32, name="b_fc_sb")
            nc.sync.dma_start(b_fc_sb[:], b_fc)
            i_sb = constp.tile([P, P], BF16, name="i_sb")
            nc.sync.dma_start(i_sb[:], ident)
            b_out_sb = constp.tile([P, C], F32, name="b_out_sb")

            for oc in range(KC):
                ps = pps.tile([P, RPC], F32, tag="pps")
                for ko in range(KC):
                    nc.tensor.matmul(ps[:], w_pr_sb[:, oc, ko, :],
                                     yt_sb[:, ko, :],
                                     start=(ko == 0), stop=(ko == KC - 1))
                nc.vector.tensor_add(x2t_sb[:, oc, :], ps[:],
                                     xt_sb[:, oc, :])

        # ---- fc + gelu (feature-major) ----
        with tc.tile_pool(name="wfcp", bufs=4) as wfcp, \
             tc.tile_pool(name="wop", bufs=4) as wop, \
             tc.tile_pool(name="fps", bufs=3, space="PSUM") as fps:
            for mc in range(MC4):
                wt = wfcp.tile([P, KC, P], BF16, tag="wfc")
                nc.sync.dma_start(wt[:], w_fc[mc * P:(mc + 1) * P, :])
                ps = fps.tile([P, RPC], F32, tag="fps")
                for ko in range(KC):
                    nc.tensor.matmul(ps[:], wt[:, ko, :], x2t_sb[:, ko, :],
                                     start=(ko == 0), stop=(ko == KC - 1))
                nc.scalar.activation(m_sb[:, mc, :], ps[:], ACT.Gelu,
                                     bias=b_fc_sb[:, mc:mc + 1], scale=1.0)
                if mc == 0:
                    # prefetch the first w_out group tiles + b_out now so the
                    # fc->out transition never waits on the DMA FIFO
                    nc.sync.dma_start(
                        b_out_sb[:], b_out[None, :].to_broadcast((P, C)))
                    wo_pre = []
                    for g in range(2):
                        wt0 = wop.tile([P, 8, 512], BF16, tag="wo",
                                       name=f"wo_pre{g}")
                        nc.sync.dma_start(wt0[:], w_out[g * P:(g + 1) * P, :])
                        wo_pre.append(wt0)

            # ---- out matmul + residual #2 (token-major; x2 via identity) --
            with tc.tile_pool(name="ofp", bufs=3) as ofp, \
                 tc.tile_pool(name="ops", bufs=1, space="PSUM") as ops:
                for ct in range(4):
                    pss = [ops.tile([P, 512], F32, tag=f"o_ps{rb}",
                                    name=f"o_ps{rb}_{ct}")
                           for rb in range(4)]
                    for g in range(8):
                        if ct == 0 and g < 2:
                            wt = wo_pre[g]
                        else:
                            wt = wop.tile([P, 8, 512], BF16, tag="wo")
                            nc.sync.dma_start(
                                wt[:],
                                w_out[(ct * 8 + g) * P:(ct * 8 + g + 1) * P, :])
                        for rb in range(4):
                            for m8 in range(8):
                                nc.tensor.matmul(
                                    pss[rb][:],
                                    m_sb[:, g * 8 + m8, rb * P:(rb + 1) * P],
                                    wt[:, m8, :],
                                    start=(g == 0 and m8 == 0), stop=False)
                    for rb in range(4):
                        for cc in range(4):
                            nc.tensor.matmul(
                                pss[rb][:, cc * P:(cc + 1) * P],
                                x2t_sb[:, ct * 4 + cc, rb * P:(rb + 1) * P],
                                i_sb[:],
                                start=False, stop=(cc == 3),
                                skip_group_check=True)
                        of = ofp.tile([P, 512], F32, tag="of")
                        nc.vector.tensor_add(
                            of[:], pss[rb][:],
                            b_out_sb[:, ct * 512:(ct + 1) * 512])
                        nc.sync.dma_start(
                            out[rb * P:(rb + 1) * P, ct * 512:(ct + 1) * 512],
                            of[:])


def _build_mlp():
    nc = bacc.Bacc("TRN2", target_bir_lowering=False, debug=False,
                   num_devices=N_CORES)
    io = {}
    io["yt"] = nc.dram_tensor("yt", [P, KC * RPC], BF16, kind="ExternalInput")
    io["xt"] = nc.dram_tensor("xt", [P, KC * RPC], BF16, kind="ExternalInput")
    io["w_pr"] = nc.dram_tensor("w_pr", [C, C], BF16, kind="ExternalInput")
    io["w_fc"] = nc.dram_tensor("w_fc", [F4, C], BF16, kind="ExternalInput")
    io["b_fc"] = nc.dram_tensor("b_fc", [P, MC4], F32, kind="ExternalInput")
    io["w_out"] = nc.dram_tensor("w_out", [2 * C, 2 * C], BF16,
                                 kind="ExternalInput")
    io["b_out"] = nc.dram_tensor("b_out", [C], F32, kind="ExternalInput")
    io["ident"] = nc.dram_tensor("ident", [P, P], BF16, kind="ExternalInput")
    io["out"] = nc.dram_tensor("out", [RPC, C], F32, kind="ExternalOutput")
    with tile.TileContext(nc) as tc:
        _emit_mlp(nc, tc, io)
    nc.compile()
    return nc


def _get_built():
    global _BUILT
    if _BUILT is None:
        _BUILT = (_build_qkv(), _build_attn(), _build_mlp())
    return _BUILT


# ======================= Host orchestration ==================================

def _tile_km(w, nchunk):
    """[K, M] -> [M/128 * 128, K/128 * 128] tiled rows: out[fc*128+p, ko*128+f]
    = w[ko*128+p, fc*128+f]."""
    K, M = w.shape
    return np.ascontiguousarray(
        w.reshape(K // P, P, nchunk, M // nchunk)
        .transpose(2, 1, 0, 3)
        .reshape(nchunk * P, K // P * (M // nchunk)))


def _tile_xt(xt_slice):
    """[2048, 512] feature-major slice -> [128, 16*512] tiled [p, ko, t]."""
    return np.ascontiguousarray(
        xt_slice.reshape(KC, P, RPC).transpose(1, 0, 2).reshape(P, KC * RPC))


def _prep(x, ln_scale, ln_bias, w_qkv, b_qkv, w_proj, b_proj,
          w_fc, b_fc, w_out, b_out):
    bf = ml_dtypes.bfloat16
    xf = np.asarray(x, np.float32).reshape(TOK, C)
    xT_bf = np.ascontiguousarray(xf.T).astype(bf)        # [C, TOK]

    # Fold LN affine into the QKV projection (exact, in float64).
    w64 = np.asarray(w_qkv, np.float64)
    w_eff = np.asarray(ln_scale, np.float64)[:, None] * w64
    b_eff = np.asarray(b_qkv, np.float64) + np.asarray(ln_bias,
                                                       np.float64) @ w64
    wqk = np.concatenate([w_eff[:, :C], w_eff[:, C:2 * C]],
                         axis=1).astype(np.float32).astype(bf)   # [C, 2C]
    wv = w_eff[:, 2 * C:].astype(np.float32).astype(bf)          # [C, C]
    bqk = np.concatenate([b_eff[:C], b_eff[C:2 * C]]).astype(np.float32)
    bv = np.ascontiguousarray(b_eff[2 * C:].astype(np.float32))

    w_qk_t = _tile_km(wqk, 32)                                   # [4096, 2048]
    w_v_t = np.ascontiguousarray(
        wv.reshape(KC, P, C).transpose(1, 0, 2).reshape(P, KC * C))
    b_qk_t = np.ascontiguousarray(bqk.reshape(32, P).T)          # [128, 32]
    cs_qk_t = np.ascontiguousarray(
        wqk.astype(np.float32).sum(0).reshape(32, P).T)          # [128, 32]

    xts = [_tile_xt(xT_bf[:, i * RPC:(i + 1) * RPC]) for i in range(N_CORES)]

    in1 = [{"xt": xts[i], "w_qk": w_qk_t, "w_v": w_v_t,
            "b_qk": b_qk_t, "cs_qk": cs_qk_t, "b_v": bv}
           for i in range(N_CORES)]

    w_pr_t = _tile_km(np.asarray(w_proj, np.float32).astype(bf), KC)
    xT3_bf = np.ascontiguousarray(
        xf.T + np.asarray(b_proj, np.float32)[:, None]).astype(bf)
    xts3 = [_tile_xt(xT3_bf[:, i * RPC:(i + 1) * RPC])
            for i in range(N_CORES)]
    w_fc_t = _tile_km(np.asarray(w_fc, np.float32).astype(bf), MC4)
    b_fc_t = np.ascontiguousarray(
        np.asarray(b_fc, np.float32).reshape(MC4, P).T)
    # w_out [8192, 2048] -> [32*128, 4096]: [ct*8+g][p][m*512+f]
    w_out_t = np.ascontiguousarray(
        np.asarray(w_out, np.float32).astype(bf)
        .reshape(8, 8, P, 4, 512).transpose(3, 0, 2, 1, 4)
        .reshape(32 * P, 4096))
    b_out_f = np.ascontiguousarray(np.asarray(b_out, np.float32))
    ident = np.eye(P, dtype=np.float32).astype(bf)
    cmask = np.triu(np.ones((P, P), np.float32)).astype(bf)
    in3_common = {"w_pr": w_pr_t, "w_fc": w_fc_t,
                  "b_fc": b_fc_t, "w_out": w_out_t, "b_out": b_out_f,
                  "ident": ident}
    return xts3, in1, in3_common, cmask


_LAUNCHERS = None  # test hook: list of 3 callables (in_maps, kwargs) -> res


class _EmuRes:
    def __init__(self, results):
        self.results = results
        self.exec_time_ns = None
        self.instructions_and_trace = None


def run(inputs, trace=False, trace_cores=None):
    """Run the three SPMD launches. Returns (out [B,T,C] f32, [res1,res2,res3])."""
    xts, in1, in3_common, cmask = _prep(**inputs)
    kwargs = {}
    if trace:
        kwargs = dict(trace=True,
                      trace_cores=trace_cores if trace_cores else [0])
    cores = list(range(N_CORES))

    if _LAUNCHERS is None:
        nc1, nc2, nc3 = _get_built()
        launch = [
            lambda ims, kw: run_bass_kernel_spmd(nc1, ims, core_ids=cores, **kw),
            lambda ims, kw: run_bass_kernel_spmd(nc2, ims, core_ids=cores, **kw),
            lambda ims, kw: run_bass_kernel_spmd(nc3, ims, core_ids=cores, **kw),
        ]
    else:
        launch = _LAUNCHERS

    res1 = launch[0](in1, kwargs)
    # qk [4096, 512] per core -> qkT_full [4096, 4096]
    qkT_full = np.concatenate(
        [np.asarray(res1.results[i]["qk"]) for i in range(N_CORES)], axis=1)
    # v [2048, 512] per core: [(tc*4+vc)*128+p, f] -> v_full [4096, 2048]
    v_full = np.concatenate(
        [np.asarray(res1.results[i]["v"])
         .reshape(4, 4, P, 512).transpose(0, 2, 1, 3).reshape(RPC, C)
         for i in range(N_CORES)], axis=0)

    qT, kT = qkT_full[:C], qkT_full[C:]
    v_r = v_full.reshape(NT, P, H, HD)
    in2 = []
    for c in range(N_CORES):
        h0 = HPC * c
        qk4 = np.ascontiguousarray(np.stack(
            [qT[h0 * HD:(h0 + 1) * HD], qT[(h0 + 1) * HD:(h0 + 2) * HD],
             kT[h0 * HD:(h0 + 1) * HD], kT[(h0 + 1) * HD:(h0 + 2) * HD]],
            axis=1).reshape(P, 4 * TOK))
        v32 = np.ascontiguousarray(
            v_r[:, :, h0:h0 + HPC, :].transpose(1, 0, 2, 3)
            .reshape(P, NT * HPC * HD))
        in2.append({"qk4": qk4, "v32": v32, "cmask": cmask})
    res2 = launch[1](in2, kwargs)
    yT_full = np.concatenate(
        [np.asarray(res2.results[i]["yt"]) for i in range(N_CORES)], axis=0)

    in3 = []
    for i in range(N_CORES):
        in3.append({"yt": _tile_xt(yT_full[:, i * RPC:(i + 1) * RPC]),
                    "xt": xts[i], **in3_common})
    res3 = launch[2](in3, kwargs)
    outf = np.concatenate(
        [np.asarray(res3.results[i]["out"]) for i in range(N_CORES)], axis=0)
    return outf.reshape(B, T, C).astype(np.float32), [res1, res2, res3]


def kernel(**inputs):
    out, _ = run(inputs, trace=False)
    return out
